# revision 1
# baseline (speedup 1.0000x reference)
"""Augmented Neural ODE kernel for 8 TRN2 NeuronCores — fp8 DoubleRow variant.

Data-parallel over the batch dim (8 batches/core -> 512 tokens/core);
state kept feature-major [STATE=128 partitions, 512 tokens] in SBUF.
Layers 1-3 (contraction 1024) run as fp8e4m3 DoubleRow matmuls: weights
interleaved [128, 2, M], K=256 per matmul, 2 MACs/cell/cycle. Per-matrix
power-of-two scales keep the fp8 range occupied; the inverse scale folds
into the tanh activation for free. Layer 0 runs in f32r straight off the
carry (K=128 can't DoubleRow; f32r streams 1 column/cycle at N=512).
The Euler carry y' = y + dt*f stays at f32r precision via an identity
matmul folded into layer 3's PSUM accumulation group (scaled by s3 so
the inverse scale cancels exactly; power-of-two makes that lossless —
a bf16 carry would accumulate ~4e-2 error over the 31 steps).
"""

import sys

if "/opt/trn_rl_repo" not in sys.path:
    sys.path.insert(0, "/opt/trn_rl_repo")

import numpy as np

B, S, DIN, DAUG = 64, 64, 64, 64
STATE = DIN + DAUG          # 128
HID = 1024
T = 32
NCORES = 8
BSHARD = B // NCORES        # 8
NTOK = BSHARD * S           # 512 tokens per core
KC = HID // 128             # 8 chunks of the hidden dim
KP = KC // 2                # 4 chunk-pairs for DoubleRow

_cached = {}


def _build(scales):
    """scales = (s1, s2, s3) power-of-two per-matrix weight scales."""
    if scales in _cached:
        return _cached[scales]
    s1, s2, s3 = scales

    import concourse.tile as tile
    from concourse import bacc, mybir

    f32 = mybir.dt.float32
    f32r = mybir.dt.float32r
    bf16 = mybir.dt.bfloat16
    fp8 = mybir.dt.float8e4
    DR = mybir.MatmulPerfMode.DoubleRow
    Tanh = mybir.ActivationFunctionType.Tanh
    Ident = mybir.ActivationFunctionType.Identity

    nc = bacc.Bacc("TRN2", target_bir_lowering=False, debug=False,
                   num_devices=NCORES)

    y0t_d = nc.dram_tensor("y0t", [DIN, NTOK], f32r, kind="ExternalInput").ap()
    laug_d = nc.dram_tensor("laug", [DIN, STATE], f32r, kind="ExternalInput").ap()
    baug_d = nc.dram_tensor("baug", [STATE, 1], f32, kind="ExternalInput").ap()
    w0t_d = nc.dram_tensor("w0t", [STATE, HID], f32r, kind="ExternalInput").ap()
    w1t_d = nc.dram_tensor("w1t", [KC, 128, HID], fp8, kind="ExternalInput").ap()
    w2t_d = nc.dram_tensor("w2t", [KC, 128, HID], fp8, kind="ExternalInput").ap()
    w3t_d = nc.dram_tensor("w3t", [KC, 128, STATE], fp8, kind="ExternalInput").ap()
    b0_d = nc.dram_tensor("b0", [128, KC], f32, kind="ExternalInput").ap()
    b1_d = nc.dram_tensor("b1", [128, KC], f32, kind="ExternalInput").ap()
    b2_d = nc.dram_tensor("b2", [128, KC], f32, kind="ExternalInput").ap()
    b3dt_d = nc.dram_tensor("b3dt", [STATE, 1], f32, kind="ExternalInput").ap()
    idt_d = nc.dram_tensor("idt", [STATE, STATE], f32r, kind="ExternalInput").ap()
    out_d = nc.dram_tensor("out", [DIN, NTOK], f32r, kind="ExternalOutput").ap()

    with tile.TileContext(nc) as tc:
        with tc.tile_pool(name="wpool", bufs=1) as wpool, \
             tc.tile_pool(name="hpool", bufs=12) as hpool, \
             tc.tile_pool(name="ypool", bufs=2) as ypool, \
             tc.tile_pool(name="pspool", bufs=8, space="PSUM") as pspool:

            w0t = wpool.tile([128, HID], f32r)
            nc.sync.dma_start(w0t[:], w0t_d[:])
            laug = wpool.tile([DIN, STATE], f32r)
            nc.sync.dma_start(laug[:], laug_d[:])
            y0t = wpool.tile([DIN, NTOK], f32r)
            nc.sync.dma_start(y0t[:], y0t_d[:])

            w1t = wpool.tile([128, KC, HID], fp8)
            w2t = wpool.tile([128, KC, HID], fp8)
            w3t = wpool.tile([128, KC, STATE], fp8)
            for g in range(KC):
                nc.gpsimd.dma_start(w1t[:, g, :], w1t_d[g])
            for g in range(KC):
                nc.scalar.dma_start(w2t[:, g, :], w2t_d[g])
            for g in range(KC):
                nc.gpsimd.dma_start(w3t[:, g, :], w3t_d[g])
            idt = wpool.tile([128, STATE], f32r)
            nc.scalar.dma_start(idt[:], idt_d[:])
            b0 = wpool.tile([128, KC], f32)
            nc.sync.dma_start(b0[:], b0_d[:])
            b1 = wpool.tile([128, KC], f32)
            nc.sync.dma_start(b1[:], b1_d[:])
            b2 = wpool.tile([128, KC], f32)
            nc.sync.dma_start(b2[:], b2_d[:])
            baug = wpool.tile([128, 1], f32)
            nc.sync.dma_start(baug[:], baug_d[:])
            b3dt = wpool.tile([128, 1], f32)
            nc.sync.dma_start(b3dt[:], b3dt_d[:])

            # augment: y = [y0; W_aug y0 + b_aug]   (K = 64, one-time)
            ps = pspool.tile([128, NTOK], f32, tag="ps")
            nc.tensor.matmul(ps[:], lhsT=laug[:], rhs=y0t[:],
                             start=True, stop=True)
            y = ypool.tile([128, NTOK], f32r, tag="y")
            nc.scalar.activation(y[:], ps[:], Ident, bias=baug[:, 0:1])

            for _step in range(T - 1):
                # layer 0: f32r (same 1 cycle/row at N=512), straight off
                # the carry y — no bf16 shadow state needed
                h0 = [hpool.tile([128, 2, NTOK], fp8, tag="h", name=f"h0_{_step}_{i}")
                      for i in range(KP)]
                for m in range(KC):
                    ps = pspool.tile([128, NTOK], f32, tag="ps")
                    nc.tensor.matmul(ps[:], lhsT=w0t[:, m * 128:(m + 1) * 128],
                                     rhs=y[:], start=True, stop=True)
                    nc.scalar.activation(h0[m // 2][:, m % 2, :], ps[:], Tanh,
                                         bias=b0[:, m:m + 1])
                # layer 1: fp8 DoubleRow, K=256 per matmul
                h1 = [hpool.tile([128, 2, NTOK], fp8, tag="h", name=f"h1_{_step}_{i}")
                      for i in range(KP)]
                for m in range(KC):
                    ms = slice(m * 128, (m + 1) * 128)
                    ps = pspool.tile([128, NTOK], f32, tag="ps")
                    for k in range(KP):
                        nc.tensor.matmul(ps[:],
                                         lhsT=w1t[:, 2 * k:2 * k + 2, ms],
                                         rhs=h0[k][:],
                                         start=(k == 0), stop=(k == KP - 1),
                                         perf_mode=DR)
                    nc.scalar.activation(h1[m // 2][:, m % 2, :], ps[:], Tanh,
                                         bias=b1[:, m:m + 1], scale=1.0 / s1)
                # layer 2 (fp8 DR) with layer 3's DR matmuls interleaved as
                # their h2 pairs become ready, so the step tail has no
                # ACT-drain wait; the Euler carry rides the same PSUM group
                # via the s3-scaled f32r identity matmul
                h2 = [hpool.tile([128, 2, NTOK], fp8, tag="h", name=f"h2_{_step}_{i}")
                      for i in range(KP)]
                ps3 = pspool.tile([128, NTOK], f32, tag="ps", name=f"ps3_{_step}")
                nc.tensor.matmul(ps3[:], lhsT=idt[:], rhs=y[:],
                                 start=True, stop=False)
                for m in range(KC):
                    ms = slice(m * 128, (m + 1) * 128)
                    ps = pspool.tile([128, NTOK], f32, tag="ps")
                    for k in range(KP):
                        nc.tensor.matmul(ps[:],
                                         lhsT=w2t[:, 2 * k:2 * k + 2, ms],
                                         rhs=h1[k][:],
                                         start=(k == 0), stop=(k == KP - 1),
                                         perf_mode=DR)
                    nc.scalar.activation(h2[m // 2][:, m % 2, :], ps[:], Tanh,
                                         bias=b2[:, m:m + 1], scale=1.0 / s2)
                    if m == 3 or m == 5 or m == 7:
                        k = (m - 3) // 2
                        nc.tensor.matmul(ps3[:],
                                         lhsT=w3t[:, 2 * k:2 * k + 2, :],
                                         rhs=h2[k][:],
                                         start=False, stop=False,
                                         perf_mode=DR)
                nc.tensor.matmul(ps3[:], lhsT=w3t[:, 6:8, :], rhs=h2[3][:],
                                 start=False, stop=True, perf_mode=DR)
                # both state views come off the vector engine: yb (bf16, the
                # critical input of next step's layer 0) first, then the f32r
                # carry; the scalar engine stays free for layer-0 tanhs
                y = ypool.tile([128, NTOK], f32r, tag="y")
                nc.vector.tensor_scalar(y[:], ps3[:], 1.0 / s3, b3dt[:, 0:1],
                                        mybir.AluOpType.mult,
                                        mybir.AluOpType.add)

            nc.sync.dma_start(out_d[:], y[0:DIN, :])

    nc.compile()
    _cached[scales] = nc
    return nc


def _pow2_scale(W, target=224.0):
    import math
    return 2.0 ** math.floor(math.log2(target / float(np.abs(W).max())))


def _make_in_maps(y0, t, W_aug, b_aug, W0, b0, W1, b1, W2, b2, W3, b3):
    import ml_dtypes
    f = np.float32
    bf = ml_dtypes.bfloat16
    f8 = ml_dtypes.float8_e4m3
    dt = float(np.asarray(t, dtype=f)[1] - np.asarray(t, dtype=f)[0])
    W1, W2 = np.asarray(W1, f), np.asarray(W2, f)
    W3dt = dt * np.asarray(W3, f)
    s1, s2, s3 = _pow2_scale(W1), _pow2_scale(W2), _pow2_scale(W3dt)

    laug = np.concatenate([np.eye(DIN, dtype=f),
                           np.asarray(W_aug, f).T], axis=1)
    baug = np.concatenate([np.zeros(DIN, f),
                           np.asarray(b_aug, f)]).reshape(STATE, 1)
    w0t = np.ascontiguousarray(np.asarray(W0, f).T)
    w1t = np.ascontiguousarray((W1 * s1).T.reshape(KC, 128, HID)).astype(f8)
    w2t = np.ascontiguousarray((W2 * s2).T.reshape(KC, 128, HID)).astype(f8)
    w3t = np.ascontiguousarray((W3dt * s3).T.reshape(KC, 128, STATE)).astype(f8)
    b0r = np.ascontiguousarray(np.asarray(b0, f).reshape(KC, 128).T)
    b1r = np.ascontiguousarray(np.asarray(b1, f).reshape(KC, 128).T)
    b2r = np.ascontiguousarray(np.asarray(b2, f).reshape(KC, 128).T)
    b3dt = (dt * np.asarray(b3, f)).reshape(STATE, 1)
    idt = np.eye(STATE, dtype=f) * s3

    shared = dict(laug=laug, baug=baug, w0t=w0t, w1t=w1t, w2t=w2t, w3t=w3t,
                  b0=b0r, b1=b1r, b2=b2r, b3dt=b3dt, idt=idt)
    in_maps = []
    for c in range(NCORES):
        y0c = np.ascontiguousarray(
            np.asarray(y0, f)[c * BSHARD:(c + 1) * BSHARD]
            .reshape(NTOK, DIN).T)
        in_maps.append(dict(y0t=y0c, **shared))
    return in_maps, (s1, s2, s3)


def _run(inputs, trace=False, **trace_kwargs):
    from concourse.bass_utils import run_bass_kernel_spmd

    in_maps, scales = _make_in_maps(**inputs)
    nc = _build(scales)
    res = run_bass_kernel_spmd(nc, in_maps, core_ids=list(range(NCORES)),
                               trace=trace, **trace_kwargs)
    outs = [res.results[c]["out"] for c in range(NCORES)]
    full = np.concatenate(
        [o.T.reshape(BSHARD, S, DIN) for o in outs], axis=0)
    return np.ascontiguousarray(full, dtype=np.float32), res


def kernel(**inputs):
    out, _ = _run(inputs, trace=False)
    return out



# revision 18
# speedup vs baseline: 1.2466x; 1.2466x over previous
"""Augmented Neural ODE kernel for 8 TRN2 NeuronCores — fp8 DoubleRow variant.

Data-parallel over the batch dim (8 batches/core -> 512 tokens/core);
state kept feature-major [STATE=128 partitions, 512 tokens] in SBUF.
Layers 1-3 (contraction 1024) run as fp8e4m3 DoubleRow matmuls: weights
interleaved [128, 2, M], K=256 per matmul, 2 MACs/cell/cycle. Per-matrix
power-of-two scales keep the fp8 range occupied; the inverse scale folds
into the tanh activation for free.

v2 changes vs the 661us baseline (all PE-side — the PE is ~100% busy):
- Layer 0 moving operand in bf16, not f32r: f32r streams at ~1 ns/col
  (SBUF-bandwidth-bound, 4B elems) while bf16 streams 1 col/cycle at the
  warm 2.4 GHz clock (~0.45 ns/col). The carry stays f32r; only the MLP
  input view is rounded to bf16 (~2^-9 relative, far under fp8 h tiles).
- The Euler carry's identity matmul is gone. y_s = z_s + s*c with
  c = dt*b3 telescoped out of the carry: the per-step +c moves into a
  step-dependent layer-0 bias b0_s = b0 + s*W0c (host-precomputed, one
  [128, 31*8] SBUF tile), and the final +31c is one DVE op before the
  output DMA. The carry is then a pure scalar_tensor_tensor on DVE
  (zbf16 for layer 0's rhs, z f32r for the next carry).
- Layer 1 runs in two half-m phases with k-passes spread across m
  (m0..3 k0, m0..3 k1, then k2/k3, then m4..7) so the PE never waits on
  the tanh stagger of h0; layer 2 keeps the baseline m-major order with
  layer 3's DoubleRow matmuls interleaved at m=3,5,7.
- Head: weight DMAs spread across 4 queues (sync/scalar/vector/gpsimd)
  in first-use order; tail: output DMA split 4 ways.
"""

import sys

if "/opt/trn_rl_repo" not in sys.path:
    sys.path.insert(0, "/opt/trn_rl_repo")

import numpy as np

B, S, DIN, DAUG = 64, 64, 64, 64
STATE = DIN + DAUG          # 128
HID = 1024
T = 32
T1 = T - 1                  # 31 Euler steps
NCORES = 8
BSHARD = B // NCORES        # 8
NTOK = BSHARD * S           # 512 tokens per core
KC = HID // 128             # 8 chunks of the hidden dim
KP = KC // 2                # 4 chunk-pairs for DoubleRow

import os
HALF_PHASE = os.environ.get("HALF_PHASE", "1") == "1"
BF16_L0 = os.environ.get("BF16_L0", "1") == "1"
DEBUG_TAPS = os.environ.get("DEBUG_TAPS", "0") == "1"

_cached = {}


def _build(scales):
    """scales = (s1, s2, s3) power-of-two per-matrix weight scales."""
    if scales in _cached:
        return _cached[scales]
    s1, s2, s3 = scales

    import concourse.tile as tile
    from concourse import bacc, mybir

    f32 = mybir.dt.float32
    f32r = mybir.dt.float32r
    bf16 = mybir.dt.bfloat16
    fp8 = mybir.dt.float8e4
    DR = mybir.MatmulPerfMode.DoubleRow
    Tanh = mybir.ActivationFunctionType.Tanh
    Ident = mybir.ActivationFunctionType.Identity
    Mult = mybir.AluOpType.mult
    Add = mybir.AluOpType.add

    nc = bacc.Bacc("TRN2", target_bir_lowering=False, debug=False,
                   num_devices=NCORES)

    y0t_d = nc.dram_tensor("y0t", [DIN, NTOK], f32r, kind="ExternalInput").ap()
    laug_d = nc.dram_tensor("laug", [DIN, STATE], f32r, kind="ExternalInput").ap()
    baug_d = nc.dram_tensor("baug", [STATE, 1], f32, kind="ExternalInput").ap()
    w0t_d = nc.dram_tensor("w0t", [STATE, HID], bf16, kind="ExternalInput").ap()
    w1t_d = nc.dram_tensor("w1t", [KC, 128, HID], fp8, kind="ExternalInput").ap()
    w2t_d = nc.dram_tensor("w2t", [KC, 128, HID], fp8, kind="ExternalInput").ap()
    w3t_d = nc.dram_tensor("w3t", [KC, 128, STATE], fp8, kind="ExternalInput").ap()
    b0t_d = nc.dram_tensor("b0t", [128, T1 * KC], f32, kind="ExternalInput").ap()
    b1_d = nc.dram_tensor("b1", [128, KC], f32, kind="ExternalInput").ap()
    b2_d = nc.dram_tensor("b2", [128, KC], f32, kind="ExternalInput").ap()
    c31_d = nc.dram_tensor("c31", [STATE, 1], f32, kind="ExternalInput").ap()
    out_d = nc.dram_tensor("out", [DIN, NTOK], f32r, kind="ExternalOutput").ap()
    if DEBUG_TAPS:
        dzb_d = nc.dram_tensor("dzb", [STATE, NTOK], bf16, kind="ExternalOutput").ap()
        db0_d = nc.dram_tensor("db0", [128, T1 * KC], f32, kind="ExternalOutput").ap()
        dh_d = {(l, i): nc.dram_tensor(f"dh{l}_{i}", [128, 2, NTOK], fp8,
                                       kind="ExternalOutput").ap()
                for l in range(3) for i in range(KP)}
        dps3_d = nc.dram_tensor("dps3", [128, NTOK], f32, kind="ExternalOutput").ap()
        ZTAPS = (0, 1, 3, 7, 15, 30)
        dz_d = {s: nc.dram_tensor(f"dz{s}", [STATE, NTOK], f32r,
                                  kind="ExternalOutput").ap() for s in ZTAPS}

    with tile.TileContext(nc) as tc:
        with tc.tile_pool(name="wpool", bufs=1) as wpool, \
             tc.tile_pool(name="hpool", bufs=12) as hpool, \
             tc.tile_pool(name="ypool", bufs=2) as ypool, \
             tc.tile_pool(name="pspool", bufs=8, space="PSUM") as pspool:

            # ---- head DMAs, in first-use order, spread over 4 queues ----
            # sync: augment inputs + small biases
            laug = wpool.tile([DIN, STATE], f32r)
            nc.sync.dma_start(laug[:], laug_d[:])
            baug = wpool.tile([128, 1], f32)
            nc.sync.dma_start(baug[:], baug_d[:])
            b0s = wpool.tile([128, T1 * KC], f32)
            nc.sync.dma_start(b0s[:, 0:2 * KC], b0t_d[:, 0:2 * KC])
            y0t = wpool.tile([DIN, NTOK], f32r)
            nc.sync.dma_start(y0t[:], y0t_d[:])
            b1 = wpool.tile([128, KC], f32)
            nc.sync.dma_start(b1[:], b1_d[:])
            b2 = wpool.tile([128, KC], f32)
            nc.sync.dma_start(b2[:], b2_d[:])
            c31 = wpool.tile([128, 1], f32)
            nc.sync.dma_start(c31[:], c31_d[:])

            # scalar: layer-0 weights (first steady-state need, ~2us in)
            w0t = wpool.tile([128, HID], bf16)
            nc.scalar.dma_start(w0t[:], w0t_d[:])

            # layer-1/2 chunks in k-pass order, round-robin over the three
            # DMA-capable queues (~100 B/ns each once streaming)
            w1t = wpool.tile([128, KC, HID], fp8)
            w2t = wpool.tile([128, KC, HID], fp8)
            qs = [nc.gpsimd, nc.sync, nc.scalar]
            for g in range(KC):
                qs[g % 3].dma_start(w1t[:, g, :], w1t_d[g])
            for g in range(KC):
                qs[(g + 2) % 3].dma_start(w2t[:, g, :], w2t_d[g])
            w3t = wpool.tile([128, KC, STATE], fp8)
            for g in range(KC):
                nc.gpsimd.dma_start(w3t[:, g, :], w3t_d[g])
            nc.sync.dma_start(b0s[:, 2 * KC:], b0t_d[:, 2 * KC:])

            # ---- augment: z0 = [y0; W_aug y0 + b_aug] ----
            ps = pspool.tile([128, NTOK], f32, tag="ps")
            nc.tensor.matmul(ps[:], lhsT=laug[:], rhs=y0t[:],
                             start=True, stop=True)
            z = ypool.tile([128, NTOK], f32r, tag="y")
            nc.scalar.activation(z[:], ps[:], Ident, bias=baug[:, 0:1])
            zb = ypool.tile([128, NTOK], bf16, tag="yb")
            nc.scalar.activation(zb[:], ps[:], Ident, bias=baug[:, 0:1])
            if DEBUG_TAPS:
                nc.gpsimd.dma_start(dzb_d[:], zb[:])
                nc.gpsimd.dma_start(db0_d[:], b0s[:])

            for _step in range(T1):
                # layer 0: bf16 moving operand off the carry view zb
                h0 = [hpool.tile([128, 2, NTOK], fp8, tag="h", name=f"h0_{_step}_{i}")
                      for i in range(KP)]
                for m in range(KC):
                    ps = pspool.tile([128, NTOK], f32, tag="ps")
                    nc.tensor.matmul(ps[:], lhsT=w0t[:, m * 128:(m + 1) * 128],
                                     rhs=zb[:], start=True, stop=True)
                    bcol = _step * KC + m
                    nc.scalar.activation(h0[m // 2][:, m % 2, :], ps[:], Tanh,
                                         bias=b0s[:, bcol:bcol + 1])
                # layer 1: fp8 DoubleRow, K=256 per matmul; two half-m
                # phases with k spread across m so the PE never waits on
                # the h0 tanh stagger
                h1 = [hpool.tile([128, 2, NTOK], fp8, tag="h", name=f"h1_{_step}_{i}")
                      for i in range(KP)]
                if HALF_PHASE:
                    ps1 = {}
                    for half in (range(0, 4), range(4, 8)):
                        for k in (0, 1):
                            for m in half:
                                if k == 0:
                                    ps1[m] = pspool.tile([128, NTOK], f32, tag="ps",
                                                         name=f"ps1_{_step}_{m}")
                                nc.tensor.matmul(ps1[m][:],
                                                 lhsT=w1t[:, 2 * k:2 * k + 2,
                                                          m * 128:(m + 1) * 128],
                                                 rhs=h0[k][:],
                                                 start=(k == 0), stop=False,
                                                 perf_mode=DR)
                        for k in (2, 3):
                            for m in half:
                                nc.tensor.matmul(ps1[m][:],
                                                 lhsT=w1t[:, 2 * k:2 * k + 2,
                                                          m * 128:(m + 1) * 128],
                                                 rhs=h0[k][:],
                                                 start=False, stop=(k == 3),
                                                 perf_mode=DR)
                        for m in half:
                            nc.scalar.activation(h1[m // 2][:, m % 2, :], ps1[m][:],
                                                 Tanh, bias=b1[:, m:m + 1],
                                                 scale=1.0 / s1)
                else:
                    for m in range(KC):
                        ms = slice(m * 128, (m + 1) * 128)
                        ps = pspool.tile([128, NTOK], f32, tag="ps")
                        for k in range(KP):
                            nc.tensor.matmul(ps[:],
                                             lhsT=w1t[:, 2 * k:2 * k + 2, ms],
                                             rhs=h0[k][:],
                                             start=(k == 0), stop=(k == KP - 1),
                                             perf_mode=DR)
                        nc.scalar.activation(h1[m // 2][:, m % 2, :], ps[:], Tanh,
                                             bias=b1[:, m:m + 1], scale=1.0 / s1)
                # layer 2 (fp8 DR, m-major) with layer 3's DR matmuls
                # interleaved as their h2 pairs become ready
                h2 = [hpool.tile([128, 2, NTOK], fp8, tag="h", name=f"h2_{_step}_{i}")
                      for i in range(KP)]
                ps3 = None
                for m in range(KC):
                    ms = slice(m * 128, (m + 1) * 128)
                    ps = pspool.tile([128, NTOK], f32, tag="ps")
                    for k in range(KP):
                        nc.tensor.matmul(ps[:],
                                         lhsT=w2t[:, 2 * k:2 * k + 2, ms],
                                         rhs=h1[k][:],
                                         start=(k == 0), stop=(k == KP - 1),
                                         perf_mode=DR)
                    nc.scalar.activation(h2[m // 2][:, m % 2, :], ps[:], Tanh,
                                         bias=b2[:, m:m + 1], scale=1.0 / s2)
                    if m == 3 or m == 5 or m == 7:
                        k = (m - 3) // 2
                        if ps3 is None:
                            # lives in the normal ps rotation; allocated
                            # late so its slot's previous tenant is long
                            # consumed, freed by the carry stt below
                            ps3 = pspool.tile([128, NTOK], f32, tag="ps",
                                              name=f"ps3_{_step}")
                        nc.tensor.matmul(ps3[:],
                                         lhsT=w3t[:, 2 * k:2 * k + 2, :],
                                         rhs=h2[k][:],
                                         start=(k == 0), stop=False,
                                         perf_mode=DR)
                nc.tensor.matmul(ps3[:], lhsT=w3t[:, 6:8, :], rhs=h2[3][:],
                                 start=False, stop=True, perf_mode=DR)
                # Euler carry on DVE: zb (bf16, the critical input of the
                # next step's layer 0) first, then the f32r carry
                zbn = ypool.tile([128, NTOK], bf16, tag="yb", name=f"zb_{_step}")
                nc.vector.scalar_tensor_tensor(zbn[:], ps3[:], 1.0 / s3, z[:],
                                               Mult, Add)
                zn = ypool.tile([128, NTOK], f32r, tag="y", name=f"z_{_step}")
                nc.vector.scalar_tensor_tensor(zn[:], ps3[:], 1.0 / s3, z[:],
                                               Mult, Add)
                if DEBUG_TAPS and _step == 0:
                    for i in range(KP):
                        nc.gpsimd.dma_start(dh_d[(0, i)][:], h0[i][:])
                        nc.gpsimd.dma_start(dh_d[(1, i)][:], h1[i][:])
                        nc.gpsimd.dma_start(dh_d[(2, i)][:], h2[i][:])
                    ps3cp = ypool.tile([128, NTOK], f32, tag="dbgps", bufs=1)
                    nc.vector.tensor_copy(ps3cp[:], ps3[:])
                    nc.gpsimd.dma_start(dps3_d[:], ps3cp[:])
                if DEBUG_TAPS and _step in ZTAPS:
                    nc.gpsimd.dma_start(dz_d[_step][:], zn[:])
                z, zb = zn, zbn

            # out = z[:DIN] + 31*c (the telescoped dt*b3 term), split 4 ways
            yout = ypool.tile([DIN, NTOK], f32r, tag="yout", bufs=1)
            nc.vector.tensor_scalar(yout[:], z[0:DIN, :], 1.0, c31[0:DIN, 0:1],
                                    Mult, Add)
            q = DIN // 4
            for i, eng in enumerate((nc.sync, nc.scalar, nc.gpsimd, nc.gpsimd)):
                rs = slice(i * q, (i + 1) * q)
                eng.dma_start(out_d[rs, :], yout[rs, :])

    nc.compile()
    _cached[scales] = nc
    return nc


def _pow2_scale(W, target=224.0):
    import math
    return 2.0 ** math.floor(math.log2(target / float(np.abs(W).max())))


def _make_in_maps(y0, t, W_aug, b_aug, W0, b0, W1, b1, W2, b2, W3, b3):
    import ml_dtypes
    f = np.float32
    bf = ml_dtypes.bfloat16
    f8 = ml_dtypes.float8_e4m3
    dt = float(np.asarray(t, dtype=f)[1] - np.asarray(t, dtype=f)[0])
    W0, W1, W2 = np.asarray(W0, f), np.asarray(W1, f), np.asarray(W2, f)
    W3dt = dt * np.asarray(W3, f)
    s1, s2, s3 = _pow2_scale(W1), _pow2_scale(W2), _pow2_scale(W3dt)

    laug = np.concatenate([np.eye(DIN, dtype=f),
                           np.asarray(W_aug, f).T], axis=1)
    baug = np.concatenate([np.zeros(DIN, f),
                           np.asarray(b_aug, f)]).reshape(STATE, 1)
    w0t = np.ascontiguousarray(W0.T).astype(bf)
    w1t = np.ascontiguousarray((W1 * s1).T.reshape(KC, 128, HID)).astype(f8)
    w2t = np.ascontiguousarray((W2 * s2).T.reshape(KC, 128, HID)).astype(f8)
    w3t = np.ascontiguousarray((W3dt * s3).T.reshape(KC, 128, STATE)).astype(f8)

    # telescoped carry: y_s = z_s + s*c with c = dt*b3; layer 0 sees
    # b0_s = b0 + s*(W0 c), and the final output adds back 31*c
    c = dt * np.asarray(b3, f)                       # [STATE]
    W0c = W0 @ c                                     # [HID]
    b0_np = np.asarray(b0, f)
    b0t = np.concatenate(
        [(b0_np + s * W0c).reshape(KC, 128).T for s in range(T1)],
        axis=1)                                      # [128, 31*KC]
    b0t = np.ascontiguousarray(b0t)
    b1r = np.ascontiguousarray(np.asarray(b1, f).reshape(KC, 128).T)
    b2r = np.ascontiguousarray(np.asarray(b2, f).reshape(KC, 128).T)
    c31 = (T1 * c).reshape(STATE, 1).astype(f)

    shared = dict(laug=laug, baug=baug, w0t=w0t, w1t=w1t, w2t=w2t, w3t=w3t,
                  b0t=b0t, b1=b1r, b2=b2r, c31=c31)
    in_maps = []
    for cix in range(NCORES):
        y0c = np.ascontiguousarray(
            np.asarray(y0, f)[cix * BSHARD:(cix + 1) * BSHARD]
            .reshape(NTOK, DIN).T)
        in_maps.append(dict(y0t=y0c, **shared))
    return in_maps, (s1, s2, s3)


def _run(inputs, trace=False, **trace_kwargs):
    from concourse.bass_utils import run_bass_kernel_spmd

    in_maps, scales = _make_in_maps(**inputs)
    nc = _build(scales)
    res = run_bass_kernel_spmd(nc, in_maps, core_ids=list(range(NCORES)),
                               trace=trace, **trace_kwargs)
    outs = [res.results[c]["out"] for c in range(NCORES)]
    full = np.concatenate(
        [o.T.reshape(BSHARD, S, DIN) for o in outs], axis=0)
    return np.ascontiguousarray(full, dtype=np.float32), res


def kernel(**inputs):
    out, _ = _run(inputs, trace=False)
    return out
